# revision 35
# baseline (speedup 1.0000x reference)
"""Trainium2 Bass kernel for nn_Encoder_8589935264 (v6).

Architecture (8 NeuronCores, one SPMD NEFF):
  1. Conv front-end is data-parallel over the 100 patches (13-patch shards,
     padded to 104).  conv1 runs from host-side im2col; conv2..5 build full-K
     im2col on device via SBUF->SBUF DMAs (activations stored flat with the
     output width padded to 18, so each kernel shift is one contiguous run
     across all patches - single-descriptor DMAs); conv6..10 run as shifted
     matmuls.  Conv weights arrive in two small packed tensors that are
     first in DMA queue order so conv starts right after launch.
  2. Transformer weight prefetch is gated on conv5's output by 1-element
     copies: otherwise ~12MB of descriptors enqueue at t=0 and starve the
     conv im2col DMAs in the shared SDMA engines.
  3. The 12-layer transformer is replicated on every core (cross-core tensor
     parallelism needs 2 collectives/layer at a ~10-25us floor each - more
     than the whole compute).  The [128,4,13] AllGather assembles h once.
     Wq/Wk/W1 are fp8-e3m4 (x64 scale; score scale and 1/64 factors folded
     into the softmax-exp scale / relu bias / residual add), cutting DMA
     bytes and LDWEIGHTS time; end-to-end error is ~1.5e-2 (< 2e-2 gate).
     V is computed with the activation chunk stationary (4 LDW + 8 N=512
     matmuls instead of 16 N=256 ones).  Per-layer emission order issues all
     heads' QK+scores (softmax tails emitted early so ACT/DVE run them
     behind later matmuls), V as PE filler, then software-pipelined
     transposes/AV (T0 T1 AV0 T2 AV1 ...) with AT evacuated on the vector
     engine; Wo/W1/W2 accumulation chains stay per-bank sequential to keep
     the PE back-to-back.  Activations stay channel-major [d,100] fp32.
"""

import numpy as np
import ml_dtypes

import concourse.bass as bass
import concourse.mybir as mybir
import concourse.tile as tile
from concourse import bacc
from concourse.bass_utils import run_bass_kernel_spmd
from concourse.masks import make_identity

BF16 = mybir.dt.bfloat16
F32 = mybir.dt.float32
F8E3 = mybir.dt.float8e3
ALU = mybir.AluOpType
AF = mybir.ActivationFunctionType
NPBF16 = ml_dtypes.bfloat16
NPF8E3 = ml_dtypes.float8_e3m4

NCORES = 8
SH = 13                 # patches per core shard
NTOK = 100
D = 512
H = 4
DK = 128
DV = 256
DFF = 2048
NL = 12
FP8_SCALE = 64.0        # weights quantized as e3m4(W * 64)

CONV_SPECS = [(8, 3, 3), (16, 8, 3), (32, 16, 3), (32, 32, 3), (64, 32, 3),
              (64, 64, 3), (128, 64, 3), (128, 128, 3), (256, 128, 3), (512, 256, 2)]
RELU_AFTER = [False, True, False, True, True, True, True, True, True, False]

# per conv layer: (ci, co, k, Hi, Wi, Ho, Wo)
GEOM = []
_hi = 20
for (o, c, k) in CONV_SPECS:
    GEOM.append((c, o, k, _hi, _hi, _hi - k + 1, _hi - k + 1))
    _hi = _hi - k + 1

AG_GROUP = list(range(NCORES))

# bias column offsets in the packed [128, 14] conv-bias tensor
_BIAS_OFF = []
_off = 0
for (o, c, k) in CONV_SPECS:
    _BIAS_OFF.append(_off)
    _off += (o + 127) // 128
N_BIAS_COLS = _off  # 14


def _im2col_chunks(li):
    """[(shift_lo, shift_hi, rows)] chunks for conv layer li (1-based, 2..5)."""
    ci, co, k, _, _, _, _ = GEOM[li - 1]
    nsh = k * k
    spc = max(1, 128 // ci)
    chunks = []
    s = 0
    while s < nsh:
        n = min(spc, nsh - s)
        chunks.append((s, s + n, n * ci))
        s += n
    return chunks


# convhead column layout: conv1..conv5 im2col-packed weights
_CH_OFF = {}
_c = 0
_CH_OFF[1] = _c; _c += 8
for _li in range(2, 6):
    _CH_OFF[_li] = _c
    _c += GEOM[_li - 1][1] * len(_im2col_chunks(_li))
CONVHEAD_COLS = _c
# convtail: conv6..10 shifted-matmul packs (baseline layout)
CONVTAIL_COLS = 576 + 1152 + 1152 + 2304 + 4096


# ---------------------------------------------------------------------------
# host-side packing
# ---------------------------------------------------------------------------

def _location_embeddings():
    pos = np.repeat(np.arange(0, 200, 20, dtype=np.float32), 10)
    k = np.arange(256, dtype=np.float32)
    inv = np.power(np.float32(10000.0), (2.0 * k / 256.0).astype(np.float32))
    ang = pos[:, None] / inv[None, :]
    return np.concatenate([np.sin(ang), np.cos(ang)], axis=1).astype(np.float32)


def _host_pack(inputs):
    f32 = np.float32
    shared = {}

    # ---- conv input patches + per-core conv1 im2col --------------------
    x = np.asarray(inputs['x'], dtype=f32)
    patches = x.reshape(3, 10, 20, 10, 20).transpose(3, 1, 0, 2, 4).reshape(NTOK, 3, 20, 20)
    ppad = np.zeros((NCORES * SH, 3, 20, 20), dtype=f32)
    ppad[:NTOK] = patches
    x1_per_core = []
    for cidx in range(NCORES):
        P = ppad[cidx * SH:(cidx + 1) * SH].transpose(1, 0, 2, 3)  # [3, SH, 20, 20]
        cols = np.empty((3, 3, 3, SH, 18, 18), dtype=f32)          # (ci,ky,kx,p,y,x)
        for ky in range(3):
            for kx in range(3):
                cols[:, ky, kx] = P[:, :, ky:ky + 18, kx:kx + 18]
        # rows ordered (s, ci) s-major to match device im2col layout:
        # transpose (ci,ky,kx,...) -> (ky,kx,ci,...)
        cols = cols.transpose(1, 2, 0, 3, 4, 5)
        x1_per_core.append(np.ascontiguousarray(
            cols.reshape(27, SH * 324)).astype(NPBF16))

    # ---- convhead: conv1..5 im2col weight packs ------------------------
    ch = np.zeros((128, CONVHEAD_COLS), dtype=f32)
    w1 = np.asarray(inputs['cw1'], f32)                 # [8,3,3,3]
    ch[:27, 0:8] = w1.transpose(2, 3, 1, 0).reshape(27, 8)
    for li in range(2, 6):
        w = np.asarray(inputs[f'cw{li}'], f32)          # [o, c, k, k]
        o, c, k, _ = w.shape
        arr = w.transpose(2, 3, 1, 0).reshape(k * k, c, o)   # [(ky kx), c, o]
        off = _CH_OFF[li]
        for (s0, s1, rows) in _im2col_chunks(li):
            blk = arr[s0:s1].reshape(rows, o)
            ch[:rows, off:off + o] = blk
            off += o
    shared['convhead'] = np.ascontiguousarray(ch).astype(NPBF16)

    # ---- convtail: conv6..10 shifted packs -----------------------------
    blocks = []
    for i in range(6, 10):
        w = np.asarray(inputs[f'cw{i}'], f32)
        o, c, k, _ = w.shape
        b = np.zeros((128, k * k * o), dtype=f32)
        b[:c] = w.transpose(1, 2, 3, 0).reshape(c, k * k * o)
        blocks.append(b)
    w10 = np.asarray(inputs['cw10'], f32)               # [512, 256, 2, 2]
    t = w10.transpose(1, 2, 3, 0).reshape(2, 128, 4, 512)      # (cic,p,s,co)
    blocks.append(t.transpose(1, 0, 2, 3).reshape(128, 4096))
    shared['convtail'] = np.ascontiguousarray(
        np.concatenate(blocks, axis=1)).astype(NPBF16)

    cb = np.zeros((128, N_BIAS_COLS), dtype=f32)
    for i, (o, c, k) in enumerate(CONV_SPECS):
        b = np.asarray(inputs[f'cb{i + 1}'], f32)
        for coc in range((o + 127) // 128):
            n = min(128, o - coc * 128)
            cb[:n, _BIAS_OFF[i] + coc] = b[coc * 128: coc * 128 + n]
    shared['cbp'] = cb

    # ---- location embedding  [128, 4, 100]  (partition-major channel) --
    le = _location_embeddings()                          # [100, 512]
    shared['locemb'] = np.ascontiguousarray(
        le.T.reshape(4, 128, NTOK).transpose(1, 0, 2)).astype(f32)

    # ---- transformer weights -------------------------------------------
    # Wq/Wk raw (score scale folded into the exp activation), fp8 e3m4 x64
    Wq = np.asarray(inputs['Wq'], f32)
    Wk = np.asarray(inputs['Wk'], f32)
    q = Wq.reshape(NL, H, 4, 128, DK).transpose(0, 3, 1, 2, 4)   # [l,p,h,kc,m]
    kk = Wk.reshape(NL, H, 4, 128, DK).transpose(0, 3, 1, 2, 4)
    wqk = np.stack([q, kk], axis=2).reshape(NL, 128, 2 * H * 4 * DK)
    shared['wqk8'] = np.ascontiguousarray(wqk * FP8_SCALE).astype(NPF8E3)

    # Wv packed for activation-stationary matmuls: [l, p(128), kc, (h v)]
    Wv = np.asarray(inputs['Wv'], f32)                   # [l,h,512,256]
    v = Wv.reshape(NL, H, 4, 128, DV).transpose(0, 3, 2, 1, 4)   # [l,p,kc,h,n]
    shared['wv16'] = np.ascontiguousarray(
        v.reshape(NL, 128, 4 * H * DV)).astype(NPBF16)           # [l,128,4096]

    Wo = np.asarray(inputs['Wo'], f32)                   # [l,1024,512]
    o = Wo.reshape(NL, 8, 128, 4, 128).transpose(0, 2, 1, 3, 4)  # [l,p,cc,oc,m]
    shared['wo16'] = np.ascontiguousarray(o.reshape(NL, 128, 8 * 4 * 128)).astype(NPBF16)

    W1 = np.asarray(inputs['W1'], f32)                   # [l,512,2048]
    a1 = W1.reshape(NL, 4, 128, 16, 128).transpose(0, 2, 1, 3, 4).reshape(NL, 128, 8192)
    shared['w18'] = np.ascontiguousarray(a1 * FP8_SCALE).astype(NPF8E3)
    W2 = np.asarray(inputs['W2'], f32)                   # [l,2048,512]
    a2 = W2.reshape(NL, 16, 128, 4, 128).transpose(0, 2, 1, 3, 4).reshape(NL, 128, 8192)
    shared['w216'] = np.ascontiguousarray(a2).astype(NPBF16)

    shared['b1p'] = np.ascontiguousarray(
        np.asarray(inputs['b1'], f32).reshape(NL, 16, 128).transpose(0, 2, 1)
        * FP8_SCALE)
    shared['b2p'] = np.ascontiguousarray(
        np.asarray(inputs['b2'], f32).reshape(NL, 4, 128).transpose(0, 2, 1))

    return shared, x1_per_core


# ---------------------------------------------------------------------------
# device kernel
# ---------------------------------------------------------------------------

def _build_nc():
    nc = bacc.Bacc("TRN2", target_bir_lowering=False, debug=False,
                   enable_asserts=False, num_devices=NCORES)

    x1 = nc.dram_tensor("x1", [27, SH * 324], BF16, kind="ExternalInput")
    convhead = nc.dram_tensor("convhead", [128, CONVHEAD_COLS], BF16, kind="ExternalInput")
    convtail = nc.dram_tensor("convtail", [128, CONVTAIL_COLS], BF16, kind="ExternalInput")
    cbp = nc.dram_tensor("cbp", [128, N_BIAS_COLS], F32, kind="ExternalInput")
    locemb = nc.dram_tensor("locemb", [128, 4, NTOK], F32, kind="ExternalInput")
    wqk8 = nc.dram_tensor("wqk8", [NL, 128, 4096], F8E3, kind="ExternalInput")
    wv16 = nc.dram_tensor("wv16", [NL, 128, 4096], BF16, kind="ExternalInput")
    wo16 = nc.dram_tensor("wo16", [NL, 128, 4096], BF16, kind="ExternalInput")
    w18 = nc.dram_tensor("w18", [NL, 128, 8192], F8E3, kind="ExternalInput")
    w216 = nc.dram_tensor("w216", [NL, 128, 8192], BF16, kind="ExternalInput")
    b1p = nc.dram_tensor("b1p", [NL, 128, 16], F32, kind="ExternalInput")
    b2p = nc.dram_tensor("b2p", [NL, 128, 4], F32, kind="ExternalInput")
    out = nc.dram_tensor("out", [NTOK, D], F32, kind="ExternalOutput")

    with tile.TileContext(nc) as tc:
        with (
            tc.tile_pool(name="consts", bufs=1) as consts,
            tc.tile_pool(name="acts", bufs=1) as acts,
            tc.tile_pool(name="conv", bufs=1) as convp,
            tc.tile_pool(name="wpool", bufs=3) as wpool,
            tc.tile_pool(name="work", bufs=2) as work,
            tc.tile_pool(name="osb", bufs=1) as osb,
            tc.tile_pool(name="upool", bufs=2) as upool,
            tc.tile_pool(name="psum", bufs=6, space="PSUM") as psum,
            tc.tile_pool(name="dram", bufs=1, space="DRAM") as dram,
        ):
            # ---------------- consts (sync queue first; x1 on scalar) ----
            ch_sb = consts.tile([128, CONVHEAD_COLS], BF16)
            nc.sync.dma_start(out=ch_sb, in_=convhead[:])
            cb_sb = consts.tile([128, N_BIAS_COLS], F32)
            nc.sync.dma_start(out=cb_sb, in_=cbp[:])
            le_sb = consts.tile([128, 4, NTOK], F32)
            nc.sync.dma_start(out=le_sb, in_=locemb[:])
            ct_sb = consts.tile([128, CONVTAIL_COLS], BF16)
            nc.sync.dma_start(out=ct_sb, in_=convtail[:], max_dma_last_dim=2048)
            x1_sb = convp.tile([27, SH, 18, 18], BF16, name="x1s", tag="x1s")
            nc.scalar.dma_start(out=x1_sb.rearrange("c p h w -> c (p h w)"), in_=x1[:])

            id128 = consts.tile([128, 128], F32, name="id128", tag="id128")
            make_identity(nc, id128[:, :])
            id100 = consts.tile([NTOK, NTOK], BF16, name="id100", tag="id100")
            make_identity(nc, id100[:, :])

            cw_sb = {}
            _o = 0
            for i in range(6, 10):
                ci, co, k, _, _, _, _ = GEOM[i - 1]
                cw_sb[i] = ct_sb[:, _o:_o + k * k * co].rearrange(
                    "p (s c) -> p s c", s=k * k); _o += k * k * co
            cw_sb[10] = ct_sb[:, _o:_o + 4096].rearrange(
                "p (a s c) -> p a s c", a=2, s=4); _o += 4096

            def bias_ap(layer_idx, coc, rows):
                return cb_sb[:rows, _BIAS_OFF[layer_idx] + coc: _BIAS_OFF[layer_idx] + coc + 1]

            # ---------------- conv1 (host im2col) ------------------------
            # A tiles for conv1..5 are stored flat [c, SH*patch + 2] with the
            # real output width padded to 18, so each im2col shift is ONE
            # contiguous run across all patches (single-descriptor DMA, cheap
            # DIRECT2D issue).  The run for shift (dy,dx) starts at dy*18+dx
            # and crosses patch boundaries; the overrun only ever lands in
            # output columns >= the real width, which no consumer reads.
            WS = 18
            A = convp.tile([8, SH * 324 + 2], BF16, name="A1", tag="Aconv", bufs=2)
            x1v = x1_sb.rearrange("c p h w -> c p (h w)")
            for p in range(SH):
                ps = psum.tile([8, 324], F32, name="ps", tag="ps")
                nc.tensor.matmul(ps, ch_sb[0:27, 0:8], x1v[:, p, :])
                nc.vector.tensor_scalar_add(A[:, p * 324:(p + 1) * 324], ps,
                                            bias_ap(0, 0, 8))

            PS_in = 324
            for li in range(2, 6):
                ci, co, k, Hi, Wi, Ho, Wo = GEOM[li - 1]
                relu = RELU_AFTER[li - 1]
                chunks = _im2col_chunks(li)
                run = Ho * WS
                ims = []
                for q, (s0, s1, rows) in enumerate(chunks):
                    # shared per-chunk-index tags: conv layers are serial, so
                    # ring reuse across layers is safe and caps SBUF usage
                    im = convp.tile([rows, SH * PS_in + 2], BF16,
                                    name=f"im{li}_{q}", tag=f"imq{q}")
                    for s in range(s0, s1):
                        dy, dx = divmod(s, k)
                        off = dy * WS + dx
                        nc.scalar.dma_start(
                            out=im[(s - s0) * ci:(s - s0 + 1) * ci,
                                   0:SH * PS_in + 2 - off],
                            in_=A[:ci, off:SH * PS_in + 2])
                    ims.append(im)
                Anew = convp.tile([co, SH * run + 2], BF16, name=f"A{li}",
                                  tag="Aconv", bufs=2)
                woff = _CH_OFF[li]
                ppc = max(1, 512 // run)          # patches per matmul chunk
                p0 = 0
                while p0 < SH:
                    pn = min(ppc, SH - p0)
                    ps = psum.tile([co, pn, run], F32, name="ps", tag="ps")
                    for q, (s0, s1, rows) in enumerate(chunks):
                        mv = ims[q][:rows, p0 * PS_in:SH * PS_in].rearrange(
                            "c (p q) -> c p q", q=PS_in)[:, 0:pn, 0:run]
                        nc.tensor.matmul(ps, ch_sb[0:rows, woff + q * co: woff + (q + 1) * co],
                                         mv,
                                         start=(q == 0), stop=(q == len(chunks) - 1))
                    psf = ps
                    dst = Anew[:, p0 * run:(p0 + pn) * run].rearrange(
                        "c (p q) -> c p q", q=run)
                    if relu:
                        nc.vector.tensor_scalar(out=dst, in0=psf,
                                                scalar1=bias_ap(li - 1, 0, co),
                                                scalar2=0.0, op0=ALU.add, op1=ALU.max)
                    else:
                        nc.vector.tensor_scalar_add(dst, psf, bias_ap(li - 1, 0, co))
                    p0 += pn
                A = Anew
                PS_in = run
            # conv5 output viewed [64, SH, 10, 18] for the shifted conv6
            A5 = A[:, 0:SH * 180].rearrange("c (p h w) -> c p h w", p=SH, w=WS)

            # ---------------- conv6..10 (shifted matmuls) ----------------
            A = None
            hconv = acts.tile([128, 4, SH], F32, name="hconv", tag="hconv")
            for li in range(6, 11):
                ci, co, k, Hi, Wi, Ho, Wo = GEOM[li - 1]
                n_cic = (ci + 127) // 128
                n_coc = (co + 127) // 128
                co_p = min(co, 128)
                relu = RELU_AFTER[li - 1]
                last = (li == 10)
                src = A5 if li == 6 else None
                if not last:
                    Anew = convp.tile([co_p, n_coc, SH, Ho, Wo], BF16, bufs=1,
                                      name=f"convA{li % 2}", tag=f"convA{li % 2}")
                npp = max(1, 512 // (Ho * Wo))
                p0 = 0
                while p0 < SH:
                    pn = min(npp, SH - p0)
                    for coc in range(n_coc):
                        ps = psum.tile([co_p, pn, Ho, Wo], F32, name="ps", tag="ps")
                        nmm = k * k * n_cic
                        mm = 0
                        for s in range(k * k):
                            dy, dx = divmod(s, k)
                            for cic in range(n_cic):
                                if li == 6:
                                    rhs = src[:, p0:p0 + pn, dy:dy + Ho, dx:dx + Wo]
                                elif n_cic == 1:
                                    rhs = A[:, 0, p0:p0 + pn, dy:dy + Ho, dx:dx + Wo]
                                else:
                                    rhs = A[:, cic, p0:p0 + pn, dy:dy + Ho, dx:dx + Wo]
                                if li == 10:
                                    lhsT = cw_sb[10][:, cic, s, coc * 128:(coc + 1) * 128]
                                else:
                                    lhsT = cw_sb[li][:ci, s, coc * 128: coc * 128 + co_p]
                                nc.tensor.matmul(ps, lhsT, rhs,
                                                 start=(mm == 0), stop=(mm == nmm - 1))
                                mm += 1
                        psf = ps.rearrange("c p h w -> c (p h w)")
                        if last:
                            dst = hconv[:, coc, p0:p0 + pn]
                            nc.vector.tensor_scalar_add(dst, psf, bias_ap(li - 1, coc, co_p))
                        else:
                            dst = Anew[:, coc, p0:p0 + pn, :, :].rearrange("c p h w -> c (p h w)")
                            if relu:
                                nc.vector.tensor_scalar(out=dst, in0=psf,
                                                        scalar1=bias_ap(li - 1, coc, co_p),
                                                        scalar2=0.0, op0=ALU.add, op1=ALU.max)
                            else:
                                nc.vector.tensor_scalar_add(dst, psf, bias_ap(li - 1, coc, co_p))
                    p0 += pn
                if not last:
                    A = Anew

            # ---------------- weight prefetch (issued before AG emission
            # so the gate copies land on the vector queue right after the
            # conv evacuations, and DMA issue order is layer-major) --------
            weights = []
            for l in range(NL):
                def gate(t):
                    # Weight DMAs of the first ring-buffer generation would
                    # otherwise enqueue megabytes of descriptors at t=0 and
                    # starve the conv im2col DMAs.  A 1-element copy reading
                    # conv5's output delays their issue until the im2col
                    # phase is done (conv6..10 need no DMA bandwidth).
                    nc.vector.tensor_copy(t[0:1, 0:1], A5[0:1, 0:1, 0:1, 0:1])

                wqk_sb = wpool.tile([128, 4096], F8E3, name="wqk", tag="wqk")
                if l < 3:
                    gate(wqk_sb)
                nc.sync.dma_start(out=wqk_sb, in_=wqk8[l])
                wv_sb = wpool.tile([128, 4, 1024], BF16, name="wv", tag="wv", bufs=2)
                if l < 2:
                    gate(wv_sb[:, 0])
                nc.sync.dma_start(out=wv_sb, in_=wv16[l])
                wo_sb = wpool.tile([128, 4096], BF16, name="wo", tag="wo", bufs=2)
                if l < 2:
                    gate(wo_sb)
                nc.gpsimd.dma_start(out=wo_sb, in_=wo16[l], max_dma_last_dim=2048)
                w1_sb = wpool.tile([128, 8192], F8E3, name="w1", tag="w1")
                if l < 3:
                    gate(w1_sb)
                nc.gpsimd.dma_start(out=w1_sb, in_=w18[l], max_dma_last_dim=4096)
                w2_sb = wpool.tile([128, 8192], BF16, name="w2", tag="w2", bufs=2)
                if l < 2:
                    gate(w2_sb)
                nc.gpsimd.dma_start(out=w2_sb, in_=w216[l], max_dma_last_dim=2048)
                b1_sb = wpool.tile([128, 16], F32, name="b1", tag="b1")
                nc.scalar.dma_start(out=b1_sb, in_=b1p[l])
                b2_sb = wpool.tile([128, 4], F32, name="b2", tag="b2")
                nc.scalar.dma_start(out=b2_sb, in_=b2p[l])
                weights.append((wqk_sb, wv_sb, wo_sb, w1_sb, w2_sb, b1_sb, b2_sb))

            # ---------------- AllGather ----------------
            inb = dram.tile([128, 4, SH], F32)
            nc.scalar.dma_start(out=inb[:], in_=hconv[:])
            agout = dram.tile([len(AG_GROUP), 128, 4, SH], F32)
            nc.gpsimd.collective_compute(
                "AllGather", ALU.bypass,
                ins=[inb[:].opt()], outs=[agout[:].opt()],
                replica_groups=[AG_GROUP],
            )

            # ---------------- assemble h (+ location embedding) ----------------
            NPAD = NCORES * SH
            hTall = acts.tile([128, 4, NPAD], F32, name="hTall", tag="hTall")
            hTball = acts.tile([128, 4, NPAD], BF16, name="hTball", tag="hTball")
            for oc in range(4):
                nc.scalar.dma_start(
                    out=hTall[:, oc, :].rearrange("p (c t) -> p c t", c=NCORES),
                    in_=agout[:, :, oc, :].rearrange("c p t -> p c t"))
            nc.vector.tensor_add(hTall[:, :, 0:NTOK], hTall[:, :, 0:NTOK], le_sb)
            nc.vector.tensor_copy(hTball[:, :, 0:NTOK], hTall[:, :, 0:NTOK])
            hT = [hTall[:, oc, 0:NTOK] for oc in range(4)]
            hTb = [hTball[:, oc, 0:NTOK] for oc in range(4)]

            EXP_SCALE = float(1.0 / (np.sqrt(np.float32(NTOK)) * FP8_SCALE * FP8_SCALE))

            # ---------------- transformer layers ----------------
            for l in range(NL):
                wqk_sb, wv_sb, wo_sb, w1_sb, w2_sb, b1_sb, b2_sb = weights[l]

                def qk_off(qk, h, kc):
                    return ((qk * H + h) * 4 + kc) * DK

                # --- emit all QK matmuls + scores (softmax chains emitted
                # immediately so ACT/DVE run them behind later QK matmuls),
                # then V as PE filler, then pipelined transposes + AV.
                qks_l = []
                Ams = []

                def emit_softmax(s_ps):
                    E = work.tile([NTOK, NTOK], BF16, name="E", tag="E")
                    ssum = work.tile([NTOK, 1], F32, name="ssum", tag="ssum")
                    nc.scalar.activation(E, s_ps, AF.Exp, scale=EXP_SCALE,
                                         accum_out=ssum)
                    rs = work.tile([NTOK, 1], F32, name="rs", tag="rs")
                    nc.vector.reciprocal(rs, ssum)
                    Am = work.tile([NTOK, NTOK], BF16, name="Am", tag="Am", bufs=4)
                    nc.vector.tensor_scalar_mul(Am, E, rs)
                    Ams.append(Am)

                for h in range(H):
                    qk_ps = psum.tile([DK, 2 * NTOK], F32, name="ps", tag="ps")
                    for kc in range(4):
                        nc.tensor.matmul(qk_ps[:, 0:NTOK],
                                         wqk_sb[:, qk_off(0, h, kc): qk_off(0, h, kc) + DK],
                                         hTb[kc], start=(kc == 0), stop=(kc == 3))
                    for kc in range(4):
                        nc.tensor.matmul(qk_ps[:, NTOK:2 * NTOK],
                                         wqk_sb[:, qk_off(1, h, kc): qk_off(1, h, kc) + DK],
                                         hTb[kc], start=(kc == 0), stop=(kc == 3))
                    qks = work.tile([DK, 2 * NTOK], BF16, name="qks", tag="qks")
                    nc.scalar.activation(qks, qk_ps, AF.Copy)
                    qks_l.append(qks)
                    if h >= 1:
                        hs = h - 1
                        s_ps = psum.tile([NTOK, NTOK], F32, name="ps", tag="ps")
                        nc.tensor.matmul(s_ps, qks_l[hs][:, 0:NTOK],
                                         qks_l[hs][:, NTOK:2 * NTOK])
                        emit_softmax(s_ps)

                # V: activation-stationary, 4 chains of 2 N=512 matmuls
                vs = osb.tile([NTOK, 4 * DV], BF16, name="vs", tag="vs")
                for j in range(2):
                    v_ps = psum.tile([NTOK, 512], F32, name="ps", tag="ps")
                    for kc in range(4):
                        nc.tensor.matmul(v_ps, hTb[kc], wv_sb[:, kc, j * 512:(j + 1) * 512],
                                         start=(kc == 0), stop=(kc == 3))
                    nc.scalar.activation(vs[:, j * 512:(j + 1) * 512], v_ps, AF.Copy)

                s_ps = psum.tile([NTOK, NTOK], F32, name="ps", tag="ps")
                nc.tensor.matmul(s_ps, qks_l[3][:, 0:NTOK], qks_l[3][:, NTOK:2 * NTOK])
                emit_softmax(s_ps)

                ots = []
                ATs = []

                def emit_T(h):
                    at_ps = psum.tile([NTOK, NTOK], BF16, name="ps_at", tag="ps_at",
                                      bufs=2)
                    nc.tensor.transpose(at_ps, Ams[h], id100)
                    AT = work.tile([NTOK, NTOK], BF16, name="AT", tag="AT")
                    # evacuate on DVE: the ACT queue is busy with Exp/V-copies
                    # here and would stall the AV matmuls behind it
                    nc.vector.tensor_copy(AT, at_ps)
                    ATs.append(AT)

                def emit_AV(h):
                    oo_ps = psum.tile([128, 2 * NTOK], F32, name="ps", tag="ps")
                    for j in range(2):
                        nc.tensor.matmul(oo_ps[:, j * NTOK:(j + 1) * NTOK],
                                         vs[:, h * 256 + j * 128: h * 256 + (j + 1) * 128],
                                         ATs[h])
                    ot = osb.tile([128, 2 * NTOK], BF16, name=f"ot{h}", tag=f"ot{h}")
                    nc.scalar.activation(ot, oo_ps, AF.Copy)
                    ots.extend([ot[:, 0:NTOK], ot[:, NTOK:2 * NTOK]])

                emit_T(0)
                emit_T(1)
                emit_AV(0)
                emit_T(2)
                emit_AV(1)
                emit_T(3)
                emit_AV(2)
                emit_AV(3)

                # sequential per-oc accumulation chains: consecutive matmuls
                # hit the same PSUM bank, which keeps the PE back-to-back
                # (interleaving banks measurably stalls it - E57 bank cycling)
                for oc in range(4):
                    z_ps = psum.tile([128, NTOK], F32, name="ps", tag="ps")
                    for cc in range(8):
                        ooff = (cc * 4 + oc) * 128
                        nc.tensor.matmul(z_ps, wo_sb[:, ooff: ooff + 128], ots[cc],
                                         start=(cc == 0), stop=(cc == 7))
                    nc.vector.tensor_add(hT[oc], hT[oc], z_ps)
                    if oc < 2:
                        nc.gpsimd.tensor_copy(hTb[oc], hT[oc])
                    else:
                        nc.vector.tensor_copy(hTb[oc], hT[oc])

                us = []
                for fc in range(16):
                    u_ps = psum.tile([128, NTOK], F32, name="ps", tag="ps")
                    for kc in range(4):
                        w1off = (kc * 16 + fc) * 128
                        nc.tensor.matmul(u_ps, w1_sb[:, w1off: w1off + 128], hTb[kc],
                                         start=(kc == 0), stop=(kc == 3))
                    u = upool.tile([128, NTOK], BF16, name=f"u{fc}", tag=f"u{fc}")
                    nc.vector.tensor_scalar(out=u, in0=u_ps, scalar1=b1_sb[:, fc:fc + 1],
                                            scalar2=0.0, op0=ALU.add, op1=ALU.max)
                    us.append(u)
                for oc in range(4):
                    # fold the FFN output bias into the residual stream early
                    nc.vector.tensor_scalar_add(hT[oc], hT[oc], b2_sb[:, oc:oc + 1])
                for oc in range(4):
                    y_ps = psum.tile([128, NTOK], F32, name="ps", tag="ps")
                    for fc in range(16):
                        w2off = (fc * 4 + oc) * 128
                        nc.tensor.matmul(y_ps, w2_sb[:, w2off: w2off + 128], us[fc],
                                         start=(fc == 0), stop=(fc == 15))
                    # y is scaled by 64 (fp8 W1 scale carried through relu); undo here
                    nc.vector.scalar_tensor_tensor(hT[oc], y_ps, 1.0 / FP8_SCALE,
                                                   hT[oc], op0=ALU.mult, op1=ALU.add)
                    if oc < 2:
                        nc.gpsimd.tensor_copy(hTb[oc], hT[oc])
                    else:
                        nc.vector.tensor_copy(hTb[oc], hT[oc])

            # ---------------- output: transpose [512,100] -> [100,512] ----------------
            out_sb = acts.tile([NTOK, D], F32, name="outsb", tag="outsb")
            for oc in range(4):
                t_ps = psum.tile([NTOK, 128], F32, name="ps", tag="ps")
                nc.tensor.transpose(t_ps, hT[oc], id128)
                nc.vector.tensor_copy(out_sb[:, oc * 128:(oc + 1) * 128], t_ps)
            nc.scalar.dma_start(out=out[:], in_=out_sb)

    nc.compile()
    return nc


_NC_CACHE = None


def kernel(**inputs):
    global _NC_CACHE
    shared, x1_per_core = _host_pack(inputs)
    if _NC_CACHE is None:
        _NC_CACHE = _build_nc()
    nc = _NC_CACHE
    in_maps = []
    for cidx in range(NCORES):
        m = dict(shared)
        m['x1'] = x1_per_core[cidx]
        in_maps.append(m)
    res = run_bass_kernel_spmd(nc, in_maps, core_ids=list(range(NCORES)))
    return res.results[0]['out']


# revision 38
# speedup vs baseline: 1.0560x; 1.0560x over previous
"""Trainium2 Bass kernel for nn_Encoder_8589935264 (v6).

Architecture (8 NeuronCores, one SPMD NEFF):
  1. Conv front-end is data-parallel over the 100 patches (13-patch shards,
     padded to 104).  conv1 runs from host-side im2col; conv2..5 build full-K
     im2col on device via SBUF->SBUF DMAs (activations stored flat with the
     output width padded to 18, so each kernel shift is one contiguous run
     across all patches - single-descriptor DMAs); conv6..10 run as shifted
     matmuls.  Conv weights arrive in two small packed tensors that are
     first in DMA queue order so conv starts right after launch.
  2. Transformer weight prefetch is gated on conv5's output by 1-element
     copies: otherwise ~12MB of descriptors enqueue at t=0 and starve the
     conv im2col DMAs in the shared SDMA engines.
  3. The 12-layer transformer is replicated on every core (cross-core tensor
     parallelism needs 2 collectives/layer at a ~10-25us floor each - more
     than the whole compute).  The [128,4,13] AllGather assembles h once.
     Wq/Wk/W1 are fp8-e3m4 (x64 scale; score scale and 1/64 factors folded
     into the softmax-exp scale / relu bias / residual add), cutting DMA
     bytes and LDWEIGHTS time; end-to-end error is ~1.5e-2 (< 2e-2 gate).
     V is computed with the activation chunk stationary (4 LDW + 8 N=512
     matmuls instead of 16 N=256 ones).  Per-layer emission order issues all
     heads' QK+scores (softmax tails emitted early so ACT/DVE run them
     behind later matmuls), V as PE filler, then software-pipelined
     transposes/AV (T0 T1 AV0 T2 AV1 ...) with AT evacuated on the vector
     engine; Wo/W1/W2 accumulation chains stay per-bank sequential to keep
     the PE back-to-back.  Activations stay channel-major [d,100] fp32.
"""

import numpy as np
import ml_dtypes

import concourse.bass as bass
import concourse.mybir as mybir
import concourse.tile as tile
from concourse import bacc
from concourse.bass_utils import run_bass_kernel_spmd
from concourse.masks import make_identity

BF16 = mybir.dt.bfloat16
F32 = mybir.dt.float32
F8E3 = mybir.dt.float8e3
ALU = mybir.AluOpType
AF = mybir.ActivationFunctionType
NPBF16 = ml_dtypes.bfloat16
NPF8E3 = ml_dtypes.float8_e3m4

NCORES = 8
SH = 13                 # patches per core shard
NTOK = 100
D = 512
H = 4
DK = 128
DV = 256
DFF = 2048
NL = 12
FP8_SCALE = 64.0        # weights quantized as e3m4(W * 64)

CONV_SPECS = [(8, 3, 3), (16, 8, 3), (32, 16, 3), (32, 32, 3), (64, 32, 3),
              (64, 64, 3), (128, 64, 3), (128, 128, 3), (256, 128, 3), (512, 256, 2)]
RELU_AFTER = [False, True, False, True, True, True, True, True, True, False]

# per conv layer: (ci, co, k, Hi, Wi, Ho, Wo)
GEOM = []
_hi = 20
for (o, c, k) in CONV_SPECS:
    GEOM.append((c, o, k, _hi, _hi, _hi - k + 1, _hi - k + 1))
    _hi = _hi - k + 1

AG_GROUP = list(range(NCORES))

# bias column offsets in the packed [128, 14] conv-bias tensor
_BIAS_OFF = []
_off = 0
for (o, c, k) in CONV_SPECS:
    _BIAS_OFF.append(_off)
    _off += (o + 127) // 128
N_BIAS_COLS = _off  # 14


# convhead column layout: conv1..conv5 im2col-packed weights.
# conv2..5 weights are stored as 3 dx-chunks of [3*ci (dy-major), co]: the
# device im2col copies only the 3 dy-shifted runs; dx becomes a base-offset
# on the matmul moving AP with a 3-step accumulation.
_CH_OFF = {}
_c = 0
_CH_OFF[1] = _c; _c += 8
for _li in range(2, 6):
    _CH_OFF[_li] = _c
    _c += GEOM[_li - 1][1] * 3
CONVHEAD_COLS = _c
# convtail: conv6..10 shifted-matmul packs (baseline layout)
CONVTAIL_COLS = 576 + 1152 + 1152 + 2304 + 4096


# ---------------------------------------------------------------------------
# host-side packing
# ---------------------------------------------------------------------------

def _location_embeddings():
    pos = np.repeat(np.arange(0, 200, 20, dtype=np.float32), 10)
    k = np.arange(256, dtype=np.float32)
    inv = np.power(np.float32(10000.0), (2.0 * k / 256.0).astype(np.float32))
    ang = pos[:, None] / inv[None, :]
    return np.concatenate([np.sin(ang), np.cos(ang)], axis=1).astype(np.float32)


def _host_pack(inputs):
    f32 = np.float32
    shared = {}

    # ---- conv input patches + per-core conv1 im2col --------------------
    x = np.asarray(inputs['x'], dtype=f32)
    patches = x.reshape(3, 10, 20, 10, 20).transpose(3, 1, 0, 2, 4).reshape(NTOK, 3, 20, 20)
    ppad = np.zeros((NCORES * SH, 3, 20, 20), dtype=f32)
    ppad[:NTOK] = patches
    x1_per_core = []
    for cidx in range(NCORES):
        P = ppad[cidx * SH:(cidx + 1) * SH].transpose(1, 0, 2, 3)  # [3, SH, 20, 20]
        cols = np.empty((3, 3, 3, SH, 18, 18), dtype=f32)          # (ci,ky,kx,p,y,x)
        for ky in range(3):
            for kx in range(3):
                cols[:, ky, kx] = P[:, :, ky:ky + 18, kx:kx + 18]
        # rows ordered (s, ci) s-major to match device im2col layout:
        # transpose (ci,ky,kx,...) -> (ky,kx,ci,...)
        cols = cols.transpose(1, 2, 0, 3, 4, 5)
        x1_per_core.append(np.ascontiguousarray(
            cols.reshape(27, SH * 324)).astype(NPBF16))

    # ---- convhead: conv1..5 im2col weight packs ------------------------
    ch = np.zeros((128, CONVHEAD_COLS), dtype=f32)
    w1 = np.asarray(inputs['cw1'], f32)                 # [8,3,3,3]
    ch[:27, 0:8] = w1.transpose(2, 3, 1, 0).reshape(27, 8)
    for li in range(2, 6):
        w = np.asarray(inputs[f'cw{li}'], f32)          # [o, c, k, k]
        o, c, k, _ = w.shape
        arr = w.transpose(2, 3, 1, 0)                   # [ky, kx, c, o]
        off = _CH_OFF[li]
        for dx in range(3):
            blk = arr[:, dx].reshape(3 * c, o)          # rows (dy, c) dy-major
            ch[:3 * c, off:off + o] = blk
            off += o
    shared['convhead'] = np.ascontiguousarray(ch).astype(NPBF16)

    # ---- convtail: conv6..10 shifted packs -----------------------------
    blocks = []
    for i in range(6, 10):
        w = np.asarray(inputs[f'cw{i}'], f32)
        o, c, k, _ = w.shape
        b = np.zeros((128, k * k * o), dtype=f32)
        b[:c] = w.transpose(1, 2, 3, 0).reshape(c, k * k * o)
        blocks.append(b)
    w10 = np.asarray(inputs['cw10'], f32)               # [512, 256, 2, 2]
    t = w10.transpose(1, 2, 3, 0).reshape(2, 128, 4, 512)      # (cic,p,s,co)
    blocks.append(t.transpose(1, 0, 2, 3).reshape(128, 4096))
    shared['convtail'] = np.ascontiguousarray(
        np.concatenate(blocks, axis=1)).astype(NPBF16)

    cb = np.zeros((128, N_BIAS_COLS), dtype=f32)
    for i, (o, c, k) in enumerate(CONV_SPECS):
        b = np.asarray(inputs[f'cb{i + 1}'], f32)
        for coc in range((o + 127) // 128):
            n = min(128, o - coc * 128)
            cb[:n, _BIAS_OFF[i] + coc] = b[coc * 128: coc * 128 + n]
    shared['cbp'] = cb

    # ---- location embedding  [128, 4, 100]  (partition-major channel) --
    le = _location_embeddings()                          # [100, 512]
    shared['locemb'] = np.ascontiguousarray(
        le.T.reshape(4, 128, NTOK).transpose(1, 0, 2)).astype(f32)

    # ---- transformer weights -------------------------------------------
    # Wq/Wk raw (score scale folded into the exp activation), fp8 e3m4 x64
    Wq = np.asarray(inputs['Wq'], f32)
    Wk = np.asarray(inputs['Wk'], f32)
    q = Wq.reshape(NL, H, 4, 128, DK).transpose(0, 3, 1, 2, 4)   # [l,p,h,kc,m]
    kk = Wk.reshape(NL, H, 4, 128, DK).transpose(0, 3, 1, 2, 4)
    wqk = np.stack([q, kk], axis=2).reshape(NL, 128, 2 * H * 4 * DK)
    shared['wqk8'] = np.ascontiguousarray(wqk * FP8_SCALE).astype(NPF8E3)

    # Wv packed for activation-stationary matmuls: [l, p(128), kc, (h v)]
    Wv = np.asarray(inputs['Wv'], f32)                   # [l,h,512,256]
    v = Wv.reshape(NL, H, 4, 128, DV).transpose(0, 3, 2, 1, 4)   # [l,p,kc,h,n]
    shared['wv16'] = np.ascontiguousarray(
        v.reshape(NL, 128, 4 * H * DV)).astype(NPBF16)           # [l,128,4096]

    Wo = np.asarray(inputs['Wo'], f32)                   # [l,1024,512]
    o = Wo.reshape(NL, 8, 128, 4, 128).transpose(0, 2, 1, 3, 4)  # [l,p,cc,oc,m]
    shared['wo16'] = np.ascontiguousarray(o.reshape(NL, 128, 8 * 4 * 128)).astype(NPBF16)

    W1 = np.asarray(inputs['W1'], f32)                   # [l,512,2048]
    a1 = W1.reshape(NL, 4, 128, 16, 128).transpose(0, 2, 1, 3, 4).reshape(NL, 128, 8192)
    shared['w18'] = np.ascontiguousarray(a1 * FP8_SCALE).astype(NPF8E3)
    W2 = np.asarray(inputs['W2'], f32)                   # [l,2048,512]
    a2 = W2.reshape(NL, 16, 128, 4, 128).transpose(0, 2, 1, 3, 4).reshape(NL, 128, 8192)
    shared['w216'] = np.ascontiguousarray(a2).astype(NPBF16)

    shared['b1p'] = np.ascontiguousarray(
        np.asarray(inputs['b1'], f32).reshape(NL, 16, 128).transpose(0, 2, 1)
        * FP8_SCALE)
    shared['b2p'] = np.ascontiguousarray(
        np.asarray(inputs['b2'], f32).reshape(NL, 4, 128).transpose(0, 2, 1))

    return shared, x1_per_core


# ---------------------------------------------------------------------------
# device kernel
# ---------------------------------------------------------------------------

def _build_nc():
    nc = bacc.Bacc("TRN2", target_bir_lowering=False, debug=False,
                   enable_asserts=False, num_devices=NCORES)

    x1 = nc.dram_tensor("x1", [27, SH * 324], BF16, kind="ExternalInput")
    convhead = nc.dram_tensor("convhead", [128, CONVHEAD_COLS], BF16, kind="ExternalInput")
    convtail = nc.dram_tensor("convtail", [128, CONVTAIL_COLS], BF16, kind="ExternalInput")
    cbp = nc.dram_tensor("cbp", [128, N_BIAS_COLS], F32, kind="ExternalInput")
    locemb = nc.dram_tensor("locemb", [128, 4, NTOK], F32, kind="ExternalInput")
    wqk8 = nc.dram_tensor("wqk8", [NL, 128, 4096], F8E3, kind="ExternalInput")
    wv16 = nc.dram_tensor("wv16", [NL, 128, 4096], BF16, kind="ExternalInput")
    wo16 = nc.dram_tensor("wo16", [NL, 128, 4096], BF16, kind="ExternalInput")
    w18 = nc.dram_tensor("w18", [NL, 128, 8192], F8E3, kind="ExternalInput")
    w216 = nc.dram_tensor("w216", [NL, 128, 8192], BF16, kind="ExternalInput")
    b1p = nc.dram_tensor("b1p", [NL, 128, 16], F32, kind="ExternalInput")
    b2p = nc.dram_tensor("b2p", [NL, 128, 4], F32, kind="ExternalInput")
    out = nc.dram_tensor("out", [NTOK, D], F32, kind="ExternalOutput")

    with tile.TileContext(nc) as tc:
        with (
            tc.tile_pool(name="consts", bufs=1) as consts,
            tc.tile_pool(name="acts", bufs=1) as acts,
            tc.tile_pool(name="conv", bufs=1) as convp,
            tc.tile_pool(name="wpool", bufs=3) as wpool,
            tc.tile_pool(name="work", bufs=2) as work,
            tc.tile_pool(name="osb", bufs=1) as osb,
            tc.tile_pool(name="upool", bufs=2) as upool,
            tc.tile_pool(name="psum", bufs=6, space="PSUM") as psum,
            tc.tile_pool(name="dram", bufs=1, space="DRAM") as dram,
        ):
            # ---------------- consts (sync queue first; x1 on scalar) ----
            ch_sb = consts.tile([128, CONVHEAD_COLS], BF16)
            nc.sync.dma_start(out=ch_sb, in_=convhead[:])
            cb_sb = consts.tile([128, N_BIAS_COLS], F32)
            nc.sync.dma_start(out=cb_sb, in_=cbp[:])
            le_sb = consts.tile([128, 4, NTOK], F32)
            nc.sync.dma_start(out=le_sb, in_=locemb[:])
            ct_sb = consts.tile([128, CONVTAIL_COLS], BF16)
            nc.sync.dma_start(out=ct_sb, in_=convtail[:], max_dma_last_dim=2048)
            x1_sb = convp.tile([27, SH, 18, 18], BF16, name="x1s", tag="x1s")
            nc.scalar.dma_start(out=x1_sb.rearrange("c p h w -> c (p h w)"), in_=x1[:])

            id128 = consts.tile([128, 128], F32, name="id128", tag="id128")
            make_identity(nc, id128[:, :])
            id100 = consts.tile([NTOK, NTOK], BF16, name="id100", tag="id100")
            make_identity(nc, id100[:, :])

            cw_sb = {}
            _o = 0
            for i in range(6, 10):
                ci, co, k, _, _, _, _ = GEOM[i - 1]
                cw_sb[i] = ct_sb[:, _o:_o + k * k * co].rearrange(
                    "p (s c) -> p s c", s=k * k); _o += k * k * co
            cw_sb[10] = ct_sb[:, _o:_o + 4096].rearrange(
                "p (a s c) -> p a s c", a=2, s=4); _o += 4096

            def bias_ap(layer_idx, coc, rows):
                return cb_sb[:rows, _BIAS_OFF[layer_idx] + coc: _BIAS_OFF[layer_idx] + coc + 1]

            # ---------------- conv1 (host im2col) ------------------------
            # A tiles for conv1..5 are stored flat [c, SH*patch + 2] with the
            # real output width padded to 18, so each im2col shift is ONE
            # contiguous run across all patches (single-descriptor DMA, cheap
            # DIRECT2D issue).  The run for shift (dy,dx) starts at dy*18+dx
            # and crosses patch boundaries; the overrun only ever lands in
            # output columns >= the real width, which no consumer reads.
            WS = 18
            A = convp.tile([8, SH * 324 + 2], BF16, name="A1", tag="Aconv", bufs=2)
            x1v = x1_sb.rearrange("c p h w -> c p (h w)")
            for p in range(SH):
                ps = psum.tile([8, 324], F32, name="ps", tag="ps")
                nc.tensor.matmul(ps, ch_sb[0:27, 0:8], x1v[:, p, :])
                nc.vector.tensor_scalar_add(A[:, p * 324:(p + 1) * 324], ps,
                                            bias_ap(0, 0, 8))

            PS_in = 324
            for li in range(2, 6):
                ci, co, k, Hi, Wi, Ho, Wo = GEOM[li - 1]
                relu = RELU_AFTER[li - 1]
                run = Ho * WS
                # dy-only im2col: 3 contiguous-run DMAs (the per-dma_start
                # DIRECT2D issue cost ~1us dominates, so fewer is faster);
                # the dx shift rides on the matmul moving-AP base offset
                im = convp.tile([3 * ci, SH * PS_in + 2], BF16,
                                name=f"im{li}", tag="imq0")
                for dy in range(3):
                    off = dy * WS
                    nc.scalar.dma_start(
                        out=im[dy * ci:(dy + 1) * ci, 0:SH * PS_in + 2 - off],
                        in_=A[:ci, off:SH * PS_in + 2])
                Anew = convp.tile([co, SH * run + 2], BF16, name=f"A{li}",
                                  tag="Aconv", bufs=2)
                woff = _CH_OFF[li]
                ppc = max(1, 512 // run)          # patches per matmul chunk
                p0 = 0
                while p0 < SH:
                    pn = min(ppc, SH - p0)
                    ps = psum.tile([co, pn, run], F32, name="ps", tag="ps")
                    for dx in range(3):
                        mv = im[:3 * ci, dx + p0 * PS_in: dx + SH * PS_in].rearrange(
                            "c (p q) -> c p q", q=PS_in)[:, 0:pn, 0:run]
                        nc.tensor.matmul(ps, ch_sb[0:3 * ci, woff + dx * co: woff + (dx + 1) * co],
                                         mv,
                                         start=(dx == 0), stop=(dx == 2))
                    psf = ps
                    dst = Anew[:, p0 * run:(p0 + pn) * run].rearrange(
                        "c (p q) -> c p q", q=run)
                    if relu:
                        nc.vector.tensor_scalar(out=dst, in0=psf,
                                                scalar1=bias_ap(li - 1, 0, co),
                                                scalar2=0.0, op0=ALU.add, op1=ALU.max)
                    else:
                        nc.vector.tensor_scalar_add(dst, psf, bias_ap(li - 1, 0, co))
                    p0 += pn
                A = Anew
                PS_in = run
            # conv5 output viewed [64, SH, 10, 18] for the shifted conv6
            A5 = A[:, 0:SH * 180].rearrange("c (p h w) -> c p h w", p=SH, w=WS)

            # ---------------- conv6..10 (shifted matmuls) ----------------
            A = None
            hconv = acts.tile([128, 4, SH], F32, name="hconv", tag="hconv")
            for li in range(6, 11):
                ci, co, k, Hi, Wi, Ho, Wo = GEOM[li - 1]
                n_cic = (ci + 127) // 128
                n_coc = (co + 127) // 128
                co_p = min(co, 128)
                relu = RELU_AFTER[li - 1]
                last = (li == 10)
                src = A5 if li == 6 else None
                if not last:
                    Anew = convp.tile([co_p, n_coc, SH, Ho, Wo], BF16, bufs=1,
                                      name=f"convA{li % 2}", tag=f"convA{li % 2}")
                npp = max(1, 512 // (Ho * Wo))
                p0 = 0
                while p0 < SH:
                    pn = min(npp, SH - p0)
                    for coc in range(n_coc):
                        ps = psum.tile([co_p, pn, Ho, Wo], F32, name="ps", tag="ps")
                        nmm = k * k * n_cic
                        mm = 0
                        for s in range(k * k):
                            dy, dx = divmod(s, k)
                            for cic in range(n_cic):
                                if li == 6:
                                    rhs = src[:, p0:p0 + pn, dy:dy + Ho, dx:dx + Wo]
                                elif n_cic == 1:
                                    rhs = A[:, 0, p0:p0 + pn, dy:dy + Ho, dx:dx + Wo]
                                else:
                                    rhs = A[:, cic, p0:p0 + pn, dy:dy + Ho, dx:dx + Wo]
                                if li == 10:
                                    lhsT = cw_sb[10][:, cic, s, coc * 128:(coc + 1) * 128]
                                else:
                                    lhsT = cw_sb[li][:ci, s, coc * 128: coc * 128 + co_p]
                                nc.tensor.matmul(ps, lhsT, rhs,
                                                 start=(mm == 0), stop=(mm == nmm - 1))
                                mm += 1
                        psf = ps.rearrange("c p h w -> c (p h w)")
                        if last:
                            dst = hconv[:, coc, p0:p0 + pn]
                            nc.vector.tensor_scalar_add(dst, psf, bias_ap(li - 1, coc, co_p))
                        else:
                            dst = Anew[:, coc, p0:p0 + pn, :, :].rearrange("c p h w -> c (p h w)")
                            if relu:
                                nc.vector.tensor_scalar(out=dst, in0=psf,
                                                        scalar1=bias_ap(li - 1, coc, co_p),
                                                        scalar2=0.0, op0=ALU.add, op1=ALU.max)
                            else:
                                nc.vector.tensor_scalar_add(dst, psf, bias_ap(li - 1, coc, co_p))
                    p0 += pn
                if not last:
                    A = Anew

            # ---------------- weight prefetch (issued before AG emission
            # so the gate copies land on the vector queue right after the
            # conv evacuations, and DMA issue order is layer-major) --------
            weights = []
            for l in range(NL):
                def gate(t):
                    # Weight DMAs of the first ring-buffer generation would
                    # otherwise enqueue megabytes of descriptors at t=0 and
                    # starve the conv im2col DMAs.  A 1-element copy reading
                    # conv5's output delays their issue until the im2col
                    # phase is done (conv6..10 need no DMA bandwidth).
                    nc.vector.tensor_copy(t[0:1, 0:1], A5[0:1, 0:1, 0:1, 0:1])

                wqk_sb = wpool.tile([128, 4096], F8E3, name="wqk", tag="wqk")
                if l < 3:
                    gate(wqk_sb)
                nc.sync.dma_start(out=wqk_sb, in_=wqk8[l])
                wv_sb = wpool.tile([128, 4, 1024], BF16, name="wv", tag="wv", bufs=2)
                if l < 2:
                    gate(wv_sb[:, 0])
                nc.sync.dma_start(out=wv_sb, in_=wv16[l])
                wo_sb = wpool.tile([128, 4096], BF16, name="wo", tag="wo", bufs=2)
                if l < 2:
                    gate(wo_sb)
                nc.gpsimd.dma_start(out=wo_sb, in_=wo16[l], max_dma_last_dim=2048)
                w1_sb = wpool.tile([128, 8192], F8E3, name="w1", tag="w1")
                if l < 3:
                    gate(w1_sb)
                nc.gpsimd.dma_start(out=w1_sb, in_=w18[l], max_dma_last_dim=4096)
                w2_sb = wpool.tile([128, 8192], BF16, name="w2", tag="w2", bufs=2)
                if l < 2:
                    gate(w2_sb)
                nc.gpsimd.dma_start(out=w2_sb, in_=w216[l], max_dma_last_dim=2048)
                b1_sb = wpool.tile([128, 16], F32, name="b1", tag="b1")
                nc.scalar.dma_start(out=b1_sb, in_=b1p[l])
                b2_sb = wpool.tile([128, 4], F32, name="b2", tag="b2")
                nc.scalar.dma_start(out=b2_sb, in_=b2p[l])
                weights.append((wqk_sb, wv_sb, wo_sb, w1_sb, w2_sb, b1_sb, b2_sb))

            # ---------------- AllGather ----------------
            inb = dram.tile([128, 4, SH], F32)
            nc.scalar.dma_start(out=inb[:], in_=hconv[:])
            agout = dram.tile([len(AG_GROUP), 128, 4, SH], F32)
            nc.gpsimd.collective_compute(
                "AllGather", ALU.bypass,
                ins=[inb[:].opt()], outs=[agout[:].opt()],
                replica_groups=[AG_GROUP],
            )

            # ---------------- assemble h (+ location embedding) ----------------
            NPAD = NCORES * SH
            hTall = acts.tile([128, 4, NPAD], F32, name="hTall", tag="hTall")
            hTball = acts.tile([128, 4, NPAD], BF16, name="hTball", tag="hTball")
            for oc in range(4):
                nc.scalar.dma_start(
                    out=hTall[:, oc, :].rearrange("p (c t) -> p c t", c=NCORES),
                    in_=agout[:, :, oc, :].rearrange("c p t -> p c t"))
            nc.vector.tensor_add(hTall[:, :, 0:NTOK], hTall[:, :, 0:NTOK], le_sb)
            nc.vector.tensor_copy(hTball[:, :, 0:NTOK], hTall[:, :, 0:NTOK])
            hT = [hTall[:, oc, 0:NTOK] for oc in range(4)]
            hTb = [hTball[:, oc, 0:NTOK] for oc in range(4)]

            EXP_SCALE = float(1.0 / (np.sqrt(np.float32(NTOK)) * FP8_SCALE * FP8_SCALE))

            # ---------------- transformer layers ----------------
            for l in range(NL):
                wqk_sb, wv_sb, wo_sb, w1_sb, w2_sb, b1_sb, b2_sb = weights[l]

                def qk_off(qk, h, kc):
                    return ((qk * H + h) * 4 + kc) * DK

                # --- emit all QK matmuls + scores (softmax chains emitted
                # immediately so ACT/DVE run them behind later QK matmuls),
                # then V as PE filler, then pipelined transposes + AV.
                qks_l = []
                Ams = []

                def emit_softmax(s_ps):
                    E = work.tile([NTOK, NTOK], BF16, name="E", tag="E")
                    ssum = work.tile([NTOK, 1], F32, name="ssum", tag="ssum")
                    nc.scalar.activation(E, s_ps, AF.Exp, scale=EXP_SCALE,
                                         accum_out=ssum)
                    rs = work.tile([NTOK, 1], F32, name="rs", tag="rs")
                    nc.vector.reciprocal(rs, ssum)
                    Am = work.tile([NTOK, NTOK], BF16, name="Am", tag="Am", bufs=4)
                    nc.vector.tensor_scalar_mul(Am, E, rs)
                    Ams.append(Am)

                for h in range(H):
                    qk_ps = psum.tile([DK, 2 * NTOK], F32, name="ps", tag="ps")
                    for kc in range(4):
                        nc.tensor.matmul(qk_ps[:, 0:NTOK],
                                         wqk_sb[:, qk_off(0, h, kc): qk_off(0, h, kc) + DK],
                                         hTb[kc], start=(kc == 0), stop=(kc == 3))
                    for kc in range(4):
                        nc.tensor.matmul(qk_ps[:, NTOK:2 * NTOK],
                                         wqk_sb[:, qk_off(1, h, kc): qk_off(1, h, kc) + DK],
                                         hTb[kc], start=(kc == 0), stop=(kc == 3))
                    qks = work.tile([DK, 2 * NTOK], BF16, name="qks", tag="qks")
                    nc.scalar.activation(qks, qk_ps, AF.Copy)
                    qks_l.append(qks)
                    if h >= 1:
                        hs = h - 1
                        s_ps = psum.tile([NTOK, NTOK], F32, name="ps", tag="ps")
                        nc.tensor.matmul(s_ps, qks_l[hs][:, 0:NTOK],
                                         qks_l[hs][:, NTOK:2 * NTOK])
                        emit_softmax(s_ps)

                # V: activation-stationary, 4 chains of 2 N=512 matmuls
                vs = osb.tile([NTOK, 4 * DV], BF16, name="vs", tag="vs")
                for j in range(2):
                    v_ps = psum.tile([NTOK, 512], F32, name="ps", tag="ps")
                    for kc in range(4):
                        nc.tensor.matmul(v_ps, hTb[kc], wv_sb[:, kc, j * 512:(j + 1) * 512],
                                         start=(kc == 0), stop=(kc == 3))
                    nc.scalar.activation(vs[:, j * 512:(j + 1) * 512], v_ps, AF.Copy)

                s_ps = psum.tile([NTOK, NTOK], F32, name="ps", tag="ps")
                nc.tensor.matmul(s_ps, qks_l[3][:, 0:NTOK], qks_l[3][:, NTOK:2 * NTOK])
                emit_softmax(s_ps)

                ots = []
                ATs = []

                def emit_T(h):
                    at_ps = psum.tile([NTOK, NTOK], BF16, name="ps_at", tag="ps_at",
                                      bufs=2)
                    nc.tensor.transpose(at_ps, Ams[h], id100)
                    AT = work.tile([NTOK, NTOK], BF16, name="AT", tag="AT")
                    # evacuate on DVE: the ACT queue is busy with Exp/V-copies
                    # here and would stall the AV matmuls behind it
                    nc.vector.tensor_copy(AT, at_ps)
                    ATs.append(AT)

                def emit_AV(h):
                    oo_ps = psum.tile([128, 2 * NTOK], F32, name="ps", tag="ps")
                    for j in range(2):
                        nc.tensor.matmul(oo_ps[:, j * NTOK:(j + 1) * NTOK],
                                         vs[:, h * 256 + j * 128: h * 256 + (j + 1) * 128],
                                         ATs[h])
                    ot = osb.tile([128, 2 * NTOK], BF16, name=f"ot{h}", tag=f"ot{h}")
                    nc.scalar.activation(ot, oo_ps, AF.Copy)
                    ots.extend([ot[:, 0:NTOK], ot[:, NTOK:2 * NTOK]])

                emit_T(0)
                emit_T(1)
                emit_AV(0)
                emit_T(2)
                emit_AV(1)
                emit_T(3)
                emit_AV(2)
                emit_AV(3)

                # sequential per-oc accumulation chains: consecutive matmuls
                # hit the same PSUM bank, which keeps the PE back-to-back
                # (interleaving banks measurably stalls it - E57 bank cycling)
                for oc in range(4):
                    z_ps = psum.tile([128, NTOK], F32, name="ps", tag="ps")
                    for cc in range(8):
                        ooff = (cc * 4 + oc) * 128
                        nc.tensor.matmul(z_ps, wo_sb[:, ooff: ooff + 128], ots[cc],
                                         start=(cc == 0), stop=(cc == 7))
                    nc.vector.tensor_add(hT[oc], hT[oc], z_ps)
                    if oc < 2:
                        nc.gpsimd.tensor_copy(hTb[oc], hT[oc])
                    else:
                        nc.vector.tensor_copy(hTb[oc], hT[oc])

                us = []
                for fc in range(16):
                    u_ps = psum.tile([128, NTOK], F32, name="ps", tag="ps")
                    for kc in range(4):
                        w1off = (kc * 16 + fc) * 128
                        nc.tensor.matmul(u_ps, w1_sb[:, w1off: w1off + 128], hTb[kc],
                                         start=(kc == 0), stop=(kc == 3))
                    u = upool.tile([128, NTOK], BF16, name=f"u{fc}", tag=f"u{fc}")
                    nc.vector.tensor_scalar(out=u, in0=u_ps, scalar1=b1_sb[:, fc:fc + 1],
                                            scalar2=0.0, op0=ALU.add, op1=ALU.max)
                    us.append(u)
                for oc in range(4):
                    # fold the FFN output bias into the residual stream early
                    nc.vector.tensor_scalar_add(hT[oc], hT[oc], b2_sb[:, oc:oc + 1])
                for oc in range(4):
                    y_ps = psum.tile([128, NTOK], F32, name="ps", tag="ps")
                    for fc in range(16):
                        w2off = (fc * 4 + oc) * 128
                        nc.tensor.matmul(y_ps, w2_sb[:, w2off: w2off + 128], us[fc],
                                         start=(fc == 0), stop=(fc == 15))
                    # y is scaled by 64 (fp8 W1 scale carried through relu); undo here
                    nc.vector.scalar_tensor_tensor(hT[oc], y_ps, 1.0 / FP8_SCALE,
                                                   hT[oc], op0=ALU.mult, op1=ALU.add)
                    if oc < 2:
                        nc.gpsimd.tensor_copy(hTb[oc], hT[oc])
                    else:
                        nc.vector.tensor_copy(hTb[oc], hT[oc])

            # ---------------- output: transpose [512,100] -> [100,512] ----------------
            out_sb = acts.tile([NTOK, D], F32, name="outsb", tag="outsb")
            for oc in range(4):
                t_ps = psum.tile([NTOK, 128], F32, name="ps", tag="ps")
                nc.tensor.transpose(t_ps, hT[oc], id128)
                nc.vector.tensor_copy(out_sb[:, oc * 128:(oc + 1) * 128], t_ps)
            nc.scalar.dma_start(out=out[:], in_=out_sb)

    nc.compile()
    return nc


_NC_CACHE = None


def kernel(**inputs):
    global _NC_CACHE
    shared, x1_per_core = _host_pack(inputs)
    if _NC_CACHE is None:
        _NC_CACHE = _build_nc()
    nc = _NC_CACHE
    in_maps = []
    for cidx in range(NCORES):
        m = dict(shared)
        m['x1'] = x1_per_core[cidx]
        in_maps.append(m)
    res = run_bass_kernel_spmd(nc, in_maps, core_ids=list(range(NCORES)))
    return res.results[0]['out']


# revision 39
# speedup vs baseline: 1.0880x; 1.0303x over previous
"""Trainium2 Bass kernel for nn_Encoder_8589935264 (v6).

Architecture (8 NeuronCores, one SPMD NEFF):
  1. Conv front-end is data-parallel over the 100 patches (13-patch shards,
     padded to 104).  conv1 runs from host-side im2col; conv2..5 build full-K
     im2col on device via SBUF->SBUF DMAs (activations stored flat with the
     output width padded to 18, so each kernel shift is one contiguous run
     across all patches - single-descriptor DMAs); conv6..10 run as shifted
     matmuls.  Conv weights arrive in two small packed tensors that are
     first in DMA queue order so conv starts right after launch.
  2. Transformer weight prefetch is gated on conv5's output by 1-element
     copies: otherwise ~12MB of descriptors enqueue at t=0 and starve the
     conv im2col DMAs in the shared SDMA engines.
  3. The 12-layer transformer is replicated on every core (cross-core tensor
     parallelism needs 2 collectives/layer at a ~10-25us floor each - more
     than the whole compute).  The [128,4,13] AllGather assembles h once.
     Wq/Wk/W1 are fp8-e3m4 (x64 scale; score scale and 1/64 factors folded
     into the softmax-exp scale / relu bias / residual add), cutting DMA
     bytes and LDWEIGHTS time; end-to-end error is ~1.5e-2 (< 2e-2 gate).
     V is computed with the activation chunk stationary (4 LDW + 8 N=512
     matmuls instead of 16 N=256 ones).  Per-layer emission order issues all
     heads' QK+scores (softmax tails emitted early so ACT/DVE run them
     behind later matmuls), V as PE filler, then software-pipelined
     transposes/AV (T0 T1 AV0 T2 AV1 ...) with AT evacuated on the vector
     engine; Wo/W1/W2 accumulation chains stay per-bank sequential to keep
     the PE back-to-back.  Activations stay channel-major [d,100] fp32.
"""

import numpy as np
import ml_dtypes

import concourse.bass as bass
import concourse.mybir as mybir
import concourse.tile as tile
from concourse import bacc
from concourse.bass_utils import run_bass_kernel_spmd
from concourse.masks import make_identity

BF16 = mybir.dt.bfloat16
F32 = mybir.dt.float32
F8E3 = mybir.dt.float8e3
ALU = mybir.AluOpType
AF = mybir.ActivationFunctionType
NPBF16 = ml_dtypes.bfloat16
NPF8E3 = ml_dtypes.float8_e3m4

NCORES = 8
SH = 13                 # patches per core shard
NTOK = 100
D = 512
H = 4
DK = 128
DV = 256
DFF = 2048
NL = 12
FP8_SCALE = 64.0        # weights quantized as e3m4(W * 64)

CONV_SPECS = [(8, 3, 3), (16, 8, 3), (32, 16, 3), (32, 32, 3), (64, 32, 3),
              (64, 64, 3), (128, 64, 3), (128, 128, 3), (256, 128, 3), (512, 256, 2)]
RELU_AFTER = [False, True, False, True, True, True, True, True, True, False]

# per conv layer: (ci, co, k, Hi, Wi, Ho, Wo)
GEOM = []
_hi = 20
for (o, c, k) in CONV_SPECS:
    GEOM.append((c, o, k, _hi, _hi, _hi - k + 1, _hi - k + 1))
    _hi = _hi - k + 1

AG_GROUP = list(range(NCORES))

# bias column offsets in the packed [128, 14] conv-bias tensor
_BIAS_OFF = []
_off = 0
for (o, c, k) in CONV_SPECS:
    _BIAS_OFF.append(_off)
    _off += (o + 127) // 128
N_BIAS_COLS = _off  # 14


# convhead column layout: conv1..conv5 im2col-packed weights.
# conv2..5 weights are stored as 3 dx-chunks of [3*ci (dy-major), co]: the
# device im2col copies only the 3 dy-shifted runs; dx becomes a base-offset
# on the matmul moving AP with a 3-step accumulation.
_CH_OFF = {}
_c = 0
_CH_OFF[1] = _c; _c += 8
for _li in range(2, 6):
    _CH_OFF[_li] = _c
    _c += GEOM[_li - 1][1] * 3
CONVHEAD_COLS = _c
# convtail: conv6..10 shifted-matmul packs (baseline layout)
CONVTAIL_COLS = 576 + 1152 + 1152 + 2304 + 4096


# ---------------------------------------------------------------------------
# host-side packing
# ---------------------------------------------------------------------------

def _location_embeddings():
    pos = np.repeat(np.arange(0, 200, 20, dtype=np.float32), 10)
    k = np.arange(256, dtype=np.float32)
    inv = np.power(np.float32(10000.0), (2.0 * k / 256.0).astype(np.float32))
    ang = pos[:, None] / inv[None, :]
    return np.concatenate([np.sin(ang), np.cos(ang)], axis=1).astype(np.float32)


def _host_pack(inputs):
    f32 = np.float32
    shared = {}

    # ---- conv input patches + per-core conv1 im2col --------------------
    x = np.asarray(inputs['x'], dtype=f32)
    patches = x.reshape(3, 10, 20, 10, 20).transpose(3, 1, 0, 2, 4).reshape(NTOK, 3, 20, 20)
    ppad = np.zeros((NCORES * SH, 3, 20, 20), dtype=f32)
    ppad[:NTOK] = patches
    x1_per_core = []
    for cidx in range(NCORES):
        P = ppad[cidx * SH:(cidx + 1) * SH].transpose(1, 0, 2, 3)  # [3, SH, 20, 20]
        cols = np.empty((3, 3, 3, SH, 18, 18), dtype=f32)          # (ci,ky,kx,p,y,x)
        for ky in range(3):
            for kx in range(3):
                cols[:, ky, kx] = P[:, :, ky:ky + 18, kx:kx + 18]
        # rows ordered (s, ci) s-major to match device im2col layout:
        # transpose (ci,ky,kx,...) -> (ky,kx,ci,...)
        cols = cols.transpose(1, 2, 0, 3, 4, 5)
        x1_per_core.append(np.ascontiguousarray(
            cols.reshape(27, SH * 324)).astype(NPBF16))

    # ---- convhead: conv1..5 im2col weight packs ------------------------
    ch = np.zeros((128, CONVHEAD_COLS), dtype=f32)
    w1 = np.asarray(inputs['cw1'], f32)                 # [8,3,3,3]
    ch[:27, 0:8] = w1.transpose(2, 3, 1, 0).reshape(27, 8)
    for li in range(2, 6):
        w = np.asarray(inputs[f'cw{li}'], f32)          # [o, c, k, k]
        o, c, k, _ = w.shape
        arr = w.transpose(2, 3, 1, 0)                   # [ky, kx, c, o]
        off = _CH_OFF[li]
        for dx in range(3):
            blk = arr[:, dx].reshape(3 * c, o)          # rows (dy, c) dy-major
            ch[:3 * c, off:off + o] = blk
            off += o
    shared['convhead'] = np.ascontiguousarray(ch).astype(NPBF16)

    # ---- convtail: conv6..10 shifted packs -----------------------------
    blocks = []
    for i in range(6, 10):
        w = np.asarray(inputs[f'cw{i}'], f32)
        o, c, k, _ = w.shape
        b = np.zeros((128, k * k * o), dtype=f32)
        b[:c] = w.transpose(1, 2, 3, 0).reshape(c, k * k * o)
        blocks.append(b)
    w10 = np.asarray(inputs['cw10'], f32)               # [512, 256, 2, 2]
    t = w10.transpose(1, 2, 3, 0).reshape(2, 128, 4, 512)      # (cic,p,s,co)
    blocks.append(t.transpose(1, 0, 2, 3).reshape(128, 4096))
    shared['convtail'] = np.ascontiguousarray(
        np.concatenate(blocks, axis=1)).astype(NPBF16)

    cb = np.zeros((128, N_BIAS_COLS), dtype=f32)
    for i, (o, c, k) in enumerate(CONV_SPECS):
        b = np.asarray(inputs[f'cb{i + 1}'], f32)
        for coc in range((o + 127) // 128):
            n = min(128, o - coc * 128)
            cb[:n, _BIAS_OFF[i] + coc] = b[coc * 128: coc * 128 + n]
    shared['cbp'] = cb

    # ---- location embedding  [128, 4, 100]  (partition-major channel) --
    le = _location_embeddings()                          # [100, 512]
    shared['locemb'] = np.ascontiguousarray(
        le.T.reshape(4, 128, NTOK).transpose(1, 0, 2)).astype(f32)

    # ---- transformer weights -------------------------------------------
    # Wq/Wk raw (score scale folded into the exp activation), fp8 e3m4 x64
    Wq = np.asarray(inputs['Wq'], f32)
    Wk = np.asarray(inputs['Wk'], f32)
    q = Wq.reshape(NL, H, 4, 128, DK).transpose(0, 3, 1, 2, 4)   # [l,p,h,kc,m]
    kk = Wk.reshape(NL, H, 4, 128, DK).transpose(0, 3, 1, 2, 4)
    wqk = np.stack([q, kk], axis=2).reshape(NL, 128, 2 * H * 4 * DK)
    shared['wqk8'] = np.ascontiguousarray(wqk * FP8_SCALE).astype(NPF8E3)

    # Wv packed for activation-stationary matmuls: [l, p(128), kc, (h v)]
    Wv = np.asarray(inputs['Wv'], f32)                   # [l,h,512,256]
    v = Wv.reshape(NL, H, 4, 128, DV).transpose(0, 3, 2, 1, 4)   # [l,p,kc,h,n]
    shared['wv16'] = np.ascontiguousarray(
        v.reshape(NL, 128, 4 * H * DV)).astype(NPBF16)           # [l,128,4096]

    Wo = np.asarray(inputs['Wo'], f32)                   # [l,1024,512]
    o = Wo.reshape(NL, 8, 128, 4, 128).transpose(0, 2, 1, 3, 4)  # [l,p,cc,oc,m]
    shared['wo16'] = np.ascontiguousarray(o.reshape(NL, 128, 8 * 4 * 128)).astype(NPBF16)

    W1 = np.asarray(inputs['W1'], f32)                   # [l,512,2048]
    a1 = W1.reshape(NL, 4, 128, 16, 128).transpose(0, 2, 1, 3, 4).reshape(NL, 128, 8192)
    shared['w18'] = np.ascontiguousarray(a1 * FP8_SCALE).astype(NPF8E3)
    W2 = np.asarray(inputs['W2'], f32)                   # [l,2048,512]
    a2 = W2.reshape(NL, 16, 128, 4, 128).transpose(0, 2, 1, 3, 4).reshape(NL, 128, 8192)
    shared['w216'] = np.ascontiguousarray(a2).astype(NPBF16)

    shared['b1p'] = np.ascontiguousarray(
        np.asarray(inputs['b1'], f32).reshape(NL, 16, 128).transpose(0, 2, 1)
        * FP8_SCALE)
    shared['b2p'] = np.ascontiguousarray(
        np.asarray(inputs['b2'], f32).reshape(NL, 4, 128).transpose(0, 2, 1))

    return shared, x1_per_core


# ---------------------------------------------------------------------------
# device kernel
# ---------------------------------------------------------------------------

def _build_nc():
    nc = bacc.Bacc("TRN2", target_bir_lowering=False, debug=False,
                   enable_asserts=False, num_devices=NCORES)

    x1 = nc.dram_tensor("x1", [27, SH * 324], BF16, kind="ExternalInput")
    convhead = nc.dram_tensor("convhead", [128, CONVHEAD_COLS], BF16, kind="ExternalInput")
    convtail = nc.dram_tensor("convtail", [128, CONVTAIL_COLS], BF16, kind="ExternalInput")
    cbp = nc.dram_tensor("cbp", [128, N_BIAS_COLS], F32, kind="ExternalInput")
    locemb = nc.dram_tensor("locemb", [128, 4, NTOK], F32, kind="ExternalInput")
    wqk8 = nc.dram_tensor("wqk8", [NL, 128, 4096], F8E3, kind="ExternalInput")
    wv16 = nc.dram_tensor("wv16", [NL, 128, 4096], BF16, kind="ExternalInput")
    wo16 = nc.dram_tensor("wo16", [NL, 128, 4096], BF16, kind="ExternalInput")
    w18 = nc.dram_tensor("w18", [NL, 128, 8192], F8E3, kind="ExternalInput")
    w216 = nc.dram_tensor("w216", [NL, 128, 8192], BF16, kind="ExternalInput")
    b1p = nc.dram_tensor("b1p", [NL, 128, 16], F32, kind="ExternalInput")
    b2p = nc.dram_tensor("b2p", [NL, 128, 4], F32, kind="ExternalInput")
    out = nc.dram_tensor("out", [NTOK, D], F32, kind="ExternalOutput")

    with tile.TileContext(nc) as tc:
        with (
            tc.tile_pool(name="consts", bufs=1) as consts,
            tc.tile_pool(name="acts", bufs=1) as acts,
            tc.tile_pool(name="conv", bufs=1) as convp,
            tc.tile_pool(name="wpool", bufs=3) as wpool,
            tc.tile_pool(name="work", bufs=2) as work,
            tc.tile_pool(name="osb", bufs=1) as osb,
            tc.tile_pool(name="upool", bufs=2) as upool,
            tc.tile_pool(name="psum", bufs=6, space="PSUM") as psum,
            tc.tile_pool(name="dram", bufs=1, space="DRAM") as dram,
        ):
            # ---------------- consts (sync queue first; x1 on scalar) ----
            ch_sb = consts.tile([128, CONVHEAD_COLS], BF16)
            nc.sync.dma_start(out=ch_sb, in_=convhead[:])
            cb_sb = consts.tile([128, N_BIAS_COLS], F32)
            nc.sync.dma_start(out=cb_sb, in_=cbp[:])
            le_sb = consts.tile([128, 4, NTOK], F32)
            nc.sync.dma_start(out=le_sb, in_=locemb[:])
            ct_sb = consts.tile([128, CONVTAIL_COLS], BF16)
            nc.sync.dma_start(out=ct_sb, in_=convtail[:], max_dma_last_dim=2048)
            x1_sb = convp.tile([27, SH, 18, 18], BF16, name="x1s", tag="x1s")
            nc.scalar.dma_start(out=x1_sb.rearrange("c p h w -> c (p h w)"), in_=x1[:])

            id128 = consts.tile([128, 128], F32, name="id128", tag="id128")
            make_identity(nc, id128[:, :])
            id100 = consts.tile([NTOK, NTOK], BF16, name="id100", tag="id100")
            make_identity(nc, id100[:, :])

            cw_sb = {}
            _o = 0
            for i in range(6, 10):
                ci, co, k, _, _, _, _ = GEOM[i - 1]
                cw_sb[i] = ct_sb[:, _o:_o + k * k * co].rearrange(
                    "p (s c) -> p s c", s=k * k); _o += k * k * co
            cw_sb[10] = ct_sb[:, _o:_o + 4096].rearrange(
                "p (a s c) -> p a s c", a=2, s=4); _o += 4096

            def bias_ap(layer_idx, coc, rows):
                return cb_sb[:rows, _BIAS_OFF[layer_idx] + coc: _BIAS_OFF[layer_idx] + coc + 1]

            # ---------------- conv1 (host im2col) ------------------------
            # A tiles for conv1..5 are stored flat [c, SH*patch + 2] with the
            # real output width padded to 18, so each im2col shift is ONE
            # contiguous run across all patches (single-descriptor DMA, cheap
            # DIRECT2D issue).  The run for shift (dy,dx) starts at dy*18+dx
            # and crosses patch boundaries; the overrun only ever lands in
            # output columns >= the real width, which no consumer reads.
            WS = 18
            A = convp.tile([8, SH * 324 + 2], BF16, name="A1", tag="Aconv", bufs=2)
            x1v = x1_sb.rearrange("c p h w -> c p (h w)")
            for p in range(SH):
                ps = psum.tile([8, 324], F32, name="ps", tag="ps")
                nc.tensor.matmul(ps, ch_sb[0:27, 0:8], x1v[:, p, :])
                nc.vector.tensor_scalar_add(A[:, p * 324:(p + 1) * 324], ps,
                                            bias_ap(0, 0, 8))

            PS_in = 324
            for li in range(2, 6):
                ci, co, k, Hi, Wi, Ho, Wo = GEOM[li - 1]
                relu = RELU_AFTER[li - 1]
                run = Ho * WS
                # dy-only im2col: 3 contiguous-run DMAs (the per-dma_start
                # DIRECT2D issue cost ~1us dominates, so fewer is faster);
                # the dx shift rides on the matmul moving-AP base offset
                im = convp.tile([3 * ci, SH * PS_in + 2], BF16,
                                name=f"im{li}", tag="imq0")
                for dy in range(3):
                    off = dy * WS
                    nc.scalar.dma_start(
                        out=im[dy * ci:(dy + 1) * ci, 0:SH * PS_in + 2 - off],
                        in_=A[:ci, off:SH * PS_in + 2])
                Anew = convp.tile([co, SH * run + 2], BF16, name=f"A{li}",
                                  tag="Aconv", bufs=2)
                woff = _CH_OFF[li]
                ppc = max(1, 512 // run)          # patches per matmul chunk
                p0 = 0
                while p0 < SH:
                    pn = min(ppc, SH - p0)
                    ps = psum.tile([co, pn, run], F32, name="ps", tag="ps")
                    for dx in range(3):
                        mv = im[:3 * ci, dx + p0 * PS_in: dx + SH * PS_in].rearrange(
                            "c (p q) -> c p q", q=PS_in)[:, 0:pn, 0:run]
                        nc.tensor.matmul(ps, ch_sb[0:3 * ci, woff + dx * co: woff + (dx + 1) * co],
                                         mv,
                                         start=(dx == 0), stop=(dx == 2))
                    psf = ps
                    dst = Anew[:, p0 * run:(p0 + pn) * run].rearrange(
                        "c (p q) -> c p q", q=run)
                    if relu:
                        nc.vector.tensor_scalar(out=dst, in0=psf,
                                                scalar1=bias_ap(li - 1, 0, co),
                                                scalar2=0.0, op0=ALU.add, op1=ALU.max)
                    else:
                        nc.vector.tensor_scalar_add(dst, psf, bias_ap(li - 1, 0, co))
                    p0 += pn
                A = Anew
                PS_in = run
            # conv5 output viewed [64, SH, 10, 18] for the shifted conv6
            A5 = A[:, 0:SH * 180].rearrange("c (p h w) -> c p h w", p=SH, w=WS)

            # ---------------- conv6..10 (shifted matmuls) ----------------
            A = None
            hconv = acts.tile([128, 4, SH], F32, name="hconv", tag="hconv")
            for li in range(6, 11):
                ci, co, k, Hi, Wi, Ho, Wo = GEOM[li - 1]
                n_cic = (ci + 127) // 128
                n_coc = (co + 127) // 128
                co_p = min(co, 128)
                relu = RELU_AFTER[li - 1]
                last = (li == 10)
                src = A5 if li == 6 else None
                if not last:
                    Anew = convp.tile([co_p, n_coc, SH, Ho, Wo], BF16, bufs=1,
                                      name=f"convA{li % 2}", tag=f"convA{li % 2}")
                npp = max(1, 512 // (Ho * Wo))
                p0 = 0
                while p0 < SH:
                    pn = min(npp, SH - p0)
                    for coc in range(n_coc):
                        ps = psum.tile([co_p, pn, Ho, Wo], F32, name="ps", tag="ps")
                        nmm = k * k * n_cic
                        mm = 0
                        for s in range(k * k):
                            dy, dx = divmod(s, k)
                            for cic in range(n_cic):
                                if li == 6:
                                    rhs = src[:, p0:p0 + pn, dy:dy + Ho, dx:dx + Wo]
                                elif n_cic == 1:
                                    rhs = A[:, 0, p0:p0 + pn, dy:dy + Ho, dx:dx + Wo]
                                else:
                                    rhs = A[:, cic, p0:p0 + pn, dy:dy + Ho, dx:dx + Wo]
                                if li == 10:
                                    lhsT = cw_sb[10][:, cic, s, coc * 128:(coc + 1) * 128]
                                else:
                                    lhsT = cw_sb[li][:ci, s, coc * 128: coc * 128 + co_p]
                                nc.tensor.matmul(ps, lhsT, rhs,
                                                 start=(mm == 0), stop=(mm == nmm - 1))
                                mm += 1
                        psf = ps.rearrange("c p h w -> c (p h w)")
                        if last:
                            dst = hconv[:, coc, p0:p0 + pn]
                            nc.vector.tensor_scalar_add(dst, psf, bias_ap(li - 1, coc, co_p))
                        else:
                            dst = Anew[:, coc, p0:p0 + pn, :, :].rearrange("c p h w -> c (p h w)")
                            if relu:
                                nc.vector.tensor_scalar(out=dst, in0=psf,
                                                        scalar1=bias_ap(li - 1, coc, co_p),
                                                        scalar2=0.0, op0=ALU.add, op1=ALU.max)
                            else:
                                nc.vector.tensor_scalar_add(dst, psf, bias_ap(li - 1, coc, co_p))
                    p0 += pn
                if not last:
                    A = Anew

            # ---------------- weight prefetch (issued before AG emission
            # so the gate copies land on the vector queue right after the
            # conv evacuations, and DMA issue order is layer-major) --------
            weights = []
            for l in range(NL):
                def gate(t):
                    # Weight DMAs of the first ring-buffer generation would
                    # otherwise enqueue megabytes of descriptors at t=0 and
                    # starve the conv im2col DMAs.  A 1-element copy reading
                    # conv5's output delays their issue until the im2col
                    # phase is done (conv6..10 need no DMA bandwidth).
                    nc.vector.tensor_copy(t[0:1, 0:1], A5[0:1, 0:1, 0:1, 0:1])

                wqk_sb = wpool.tile([128, 4096], F8E3, name="wqk", tag="wqk")
                if l < 3:
                    gate(wqk_sb)
                nc.sync.dma_start(out=wqk_sb, in_=wqk8[l])
                wv_sb = wpool.tile([128, 4, 1024], BF16, name="wv", tag="wv", bufs=2)
                if l < 2:
                    gate(wv_sb[:, 0])
                nc.sync.dma_start(out=wv_sb, in_=wv16[l])
                wo_sb = wpool.tile([128, 4096], BF16, name="wo", tag="wo", bufs=2)
                if l < 2:
                    gate(wo_sb)
                nc.gpsimd.dma_start(out=wo_sb, in_=wo16[l], max_dma_last_dim=2048)
                w1_sb = wpool.tile([128, 8192], F8E3, name="w1", tag="w1")
                if l < 3:
                    gate(w1_sb)
                nc.gpsimd.dma_start(out=w1_sb, in_=w18[l], max_dma_last_dim=4096)
                w2_sb = wpool.tile([128, 8192], BF16, name="w2", tag="w2", bufs=2)
                if l < 2:
                    gate(w2_sb)
                nc.gpsimd.dma_start(out=w2_sb, in_=w216[l], max_dma_last_dim=2048)
                b1_sb = wpool.tile([128, 16], F32, name="b1", tag="b1")
                nc.scalar.dma_start(out=b1_sb, in_=b1p[l])
                b2_sb = wpool.tile([128, 4], F32, name="b2", tag="b2")
                nc.scalar.dma_start(out=b2_sb, in_=b2p[l])
                weights.append((wqk_sb, wv_sb, wo_sb, w1_sb, w2_sb, b1_sb, b2_sb))

            # ---------------- AllGather ----------------
            inb = dram.tile([128, 4, SH], F32)
            nc.scalar.dma_start(out=inb[:], in_=hconv[:])
            agout = dram.tile([len(AG_GROUP), 128, 4, SH], F32)
            nc.gpsimd.collective_compute(
                "AllGather", ALU.bypass,
                ins=[inb[:].opt()], outs=[agout[:].opt()],
                replica_groups=[AG_GROUP],
            )

            # ---------------- assemble h (+ location embedding) ----------------
            NPAD = NCORES * SH
            hTall = acts.tile([128, 4, NPAD], F32, name="hTall", tag="hTall")
            hTball = acts.tile([128, 4, NPAD], BF16, name="hTball", tag="hTball")
            for oc in range(4):
                nc.scalar.dma_start(
                    out=hTall[:, oc, :].rearrange("p (c t) -> p c t", c=NCORES),
                    in_=agout[:, :, oc, :].rearrange("c p t -> p c t"))
            nc.vector.tensor_add(hTall[:, :, 0:NTOK], hTall[:, :, 0:NTOK], le_sb)
            nc.vector.tensor_copy(hTball[:, :, 0:NTOK], hTall[:, :, 0:NTOK])
            hT = [hTall[:, oc, 0:NTOK] for oc in range(4)]
            hTb = [hTball[:, oc, 0:NTOK] for oc in range(4)]

            EXP_SCALE = float(1.0 / (np.sqrt(np.float32(NTOK)) * FP8_SCALE * FP8_SCALE))

            # ---------------- transformer layers ----------------
            for l in range(NL):
                wqk_sb, wv_sb, wo_sb, w1_sb, w2_sb, b1_sb, b2_sb = weights[l]

                def qk_off(qk, h, kc):
                    return ((qk * H + h) * 4 + kc) * DK

                # --- emit all QK matmuls + scores (softmax chains emitted
                # immediately so ACT/DVE run them behind later QK matmuls),
                # then V as PE filler, then pipelined transposes + AV.
                qks_l = []
                Ams = []

                def emit_softmax(s_ps):
                    E = work.tile([NTOK, NTOK], BF16, name="E", tag="E")
                    ssum = work.tile([NTOK, 1], F32, name="ssum", tag="ssum")
                    nc.scalar.activation(E, s_ps, AF.Exp, scale=EXP_SCALE,
                                         accum_out=ssum)
                    rs = work.tile([NTOK, 1], F32, name="rs", tag="rs")
                    nc.vector.reciprocal(rs, ssum)
                    Am = work.tile([NTOK, NTOK], BF16, name="Am", tag="Am", bufs=4)
                    nc.vector.tensor_scalar_mul(Am, E, rs)
                    Ams.append(Am)

                for h in range(H):
                    qk_ps = psum.tile([DK, 2 * NTOK], F32, name="ps", tag="ps")
                    for kc in range(4):
                        nc.tensor.matmul(qk_ps[:, 0:NTOK],
                                         wqk_sb[:, qk_off(0, h, kc): qk_off(0, h, kc) + DK],
                                         hTb[kc], start=(kc == 0), stop=(kc == 3))
                    for kc in range(4):
                        nc.tensor.matmul(qk_ps[:, NTOK:2 * NTOK],
                                         wqk_sb[:, qk_off(1, h, kc): qk_off(1, h, kc) + DK],
                                         hTb[kc], start=(kc == 0), stop=(kc == 3))
                    qks = work.tile([DK, 2 * NTOK], BF16, name="qks", tag="qks")
                    nc.scalar.activation(qks, qk_ps, AF.Copy)
                    qks_l.append(qks)
                    if h >= 1:
                        hs = h - 1
                        s_ps = psum.tile([NTOK, NTOK], F32, name="ps", tag="ps")
                        nc.tensor.matmul(s_ps, qks_l[hs][:, 0:NTOK],
                                         qks_l[hs][:, NTOK:2 * NTOK])
                        emit_softmax(s_ps)

                # V: activation-stationary, 4 chains of 2 N=512 matmuls
                vs = osb.tile([NTOK, 4 * DV], BF16, name="vs", tag="vs")
                for j in range(2):
                    v_ps = psum.tile([NTOK, 512], F32, name="ps", tag="ps")
                    for kc in range(4):
                        nc.tensor.matmul(v_ps, hTb[kc], wv_sb[:, kc, j * 512:(j + 1) * 512],
                                         start=(kc == 0), stop=(kc == 3))
                    nc.scalar.activation(vs[:, j * 512:(j + 1) * 512], v_ps, AF.Copy)

                s_ps = psum.tile([NTOK, NTOK], F32, name="ps", tag="ps")
                nc.tensor.matmul(s_ps, qks_l[3][:, 0:NTOK], qks_l[3][:, NTOK:2 * NTOK])
                emit_softmax(s_ps)

                ots = []
                ATs = []

                def emit_T(h):
                    at_ps = psum.tile([NTOK, NTOK], BF16, name="ps_at", tag="ps_at",
                                      bufs=2)
                    nc.tensor.transpose(at_ps, Ams[h], id100)
                    AT = work.tile([NTOK, NTOK], BF16, name="AT", tag="AT")
                    # evacuate on DVE: the ACT queue is busy with Exp/V-copies
                    # here and would stall the AV matmuls behind it
                    nc.vector.tensor_copy(AT, at_ps)
                    ATs.append(AT)

                def emit_AV(h):
                    oo_ps = psum.tile([128, 2 * NTOK], F32, name="ps", tag="ps")
                    for j in range(2):
                        nc.tensor.matmul(oo_ps[:, j * NTOK:(j + 1) * NTOK],
                                         vs[:, h * 256 + j * 128: h * 256 + (j + 1) * 128],
                                         ATs[h])
                    ot = osb.tile([128, 2 * NTOK], BF16, name=f"ot{h}", tag=f"ot{h}")
                    nc.scalar.activation(ot, oo_ps, AF.Copy)
                    ots.extend([ot[:, 0:NTOK], ot[:, NTOK:2 * NTOK]])

                emit_T(0)
                emit_T(1)
                emit_AV(0)
                emit_T(2)
                emit_AV(1)
                emit_T(3)
                emit_AV(2)
                emit_AV(3)

                # sequential per-oc accumulation chains: consecutive matmuls
                # hit the same PSUM bank, which keeps the PE back-to-back
                # (interleaving banks measurably stalls it - E57 bank cycling)
                for oc in range(4):
                    z_ps = psum.tile([128, NTOK], F32, name="ps", tag="ps")
                    for cc in range(8):
                        ooff = (cc * 4 + oc) * 128
                        nc.tensor.matmul(z_ps, wo_sb[:, ooff: ooff + 128], ots[cc],
                                         start=(cc == 0), stop=(cc == 7))
                    nc.vector.tensor_add(hT[oc], hT[oc], z_ps)
                    # casts stay on DVE: gpsimd is busy with SWDGE descriptor
                    # generation and would stall the next matmul chain
                    nc.vector.tensor_copy(hTb[oc], hT[oc])

                us = []
                for fc in range(16):
                    u_ps = psum.tile([128, NTOK], F32, name="ps", tag="ps")
                    for kc in range(4):
                        w1off = (kc * 16 + fc) * 128
                        nc.tensor.matmul(u_ps, w1_sb[:, w1off: w1off + 128], hTb[kc],
                                         start=(kc == 0), stop=(kc == 3))
                    u = upool.tile([128, NTOK], BF16, name=f"u{fc}", tag=f"u{fc}")
                    nc.vector.tensor_scalar(out=u, in0=u_ps, scalar1=b1_sb[:, fc:fc + 1],
                                            scalar2=0.0, op0=ALU.add, op1=ALU.max)
                    us.append(u)
                for oc in range(4):
                    # fold the FFN output bias into the residual stream early
                    nc.vector.tensor_scalar_add(hT[oc], hT[oc], b2_sb[:, oc:oc + 1])
                for oc in range(4):
                    y_ps = psum.tile([128, NTOK], F32, name="ps", tag="ps")
                    for fc in range(16):
                        w2off = (fc * 4 + oc) * 128
                        nc.tensor.matmul(y_ps, w2_sb[:, w2off: w2off + 128], us[fc],
                                         start=(fc == 0), stop=(fc == 15))
                    # y is scaled by 64 (fp8 W1 scale carried through relu); undo here
                    nc.vector.scalar_tensor_tensor(hT[oc], y_ps, 1.0 / FP8_SCALE,
                                                   hT[oc], op0=ALU.mult, op1=ALU.add)
                    nc.vector.tensor_copy(hTb[oc], hT[oc])

            # ---------------- output: transpose [512,100] -> [100,512] ----------------
            out_sb = acts.tile([NTOK, D], F32, name="outsb", tag="outsb")
            for oc in range(4):
                t_ps = psum.tile([NTOK, 128], F32, name="ps", tag="ps")
                nc.tensor.transpose(t_ps, hT[oc], id128)
                nc.vector.tensor_copy(out_sb[:, oc * 128:(oc + 1) * 128], t_ps)
            nc.scalar.dma_start(out=out[:], in_=out_sb)

    nc.compile()
    return nc


_NC_CACHE = None


def kernel(**inputs):
    global _NC_CACHE
    shared, x1_per_core = _host_pack(inputs)
    if _NC_CACHE is None:
        _NC_CACHE = _build_nc()
    nc = _NC_CACHE
    in_maps = []
    for cidx in range(NCORES):
        m = dict(shared)
        m['x1'] = x1_per_core[cidx]
        in_maps.append(m)
    res = run_bass_kernel_spmd(nc, in_maps, core_ids=list(range(NCORES)))
    return res.results[0]['out']
